# revision 6
# baseline (speedup 1.0000x reference)
"""LoRA layer kernel for Trainium2 (8 NeuronCores, data-parallel).

Computes out = SCALING * (x @ A^T) @ B^T for x [4, 8192, 1024],
lora_A [4, 1024], lora_B [1024, 4], SCALING = 0.25.

Strategy (per core, shard = 4096 rows x 1024 features), memory-bound:
  - Host pre-transposes and pre-rounds x to bf16 in the exact SBUF slab
    layout [slab][p][chunk][row]: every load is one DMA with 8 KiB
    per-partition contiguous lines and NO on-chip transpose. Output is
    written bf16 in a packed [slab][o2][p][j][o] layout (host un-permutes
    and upcasts). Per-core HBM traffic: 8 MiB in + 8 MiB out.
  - mm1 (rank projection): A's 4 columns are replicated into PE array
    columns {0-3, 32-35, 64-67, 96-99} with zeros between (host-prepared
    weights), so the 8 chunk-accumulation matmuls produce h^T already
    replicated at 4 PSUM partition offsets - free replication for the
    row-tiled second stage, with exact zeros elsewhere.
  - mm2: 4 concurrent row-tiled matmuls (tile_position=(32r, 0)); tile r
    reads jtile r's h^T from partitions 32r..32r+3 and streams its own
    B half, so 4 jtiles finish in ~one N=512 stream time.
  - Software pipeline: each slab's mm1 chain is split into two 4-chunk
    half-chains that sandwich the PREVIOUS slab's two mm2 waves, so the
    PE always has matmul work while wave PSUM banks are evacuated
    (bank budget: htx 2 + per-r bufs (2,2,1,1) = 8). Single-buffered
    banks are evacuated first, split across ScalarE and VectorE.
  - Loads ride the sync (HWDGE) ring (slab 0 split per chunk so mm1
    starts after 128 KiB, not 1 MiB); stores ride the gpsimd (SWDGE)
    ring, one 512 KiB store per wave.
"""

import sys

for _p in (
    "/root/.axon_site",
    "/root/.axon_site/_ro/trn_rl_repo",
    "/root/.axon_site/_ro/pypackages",
):
    if _p not in sys.path:
        sys.path.insert(0, _p)

from contextlib import ExitStack

import numpy as np
import ml_dtypes

BF16 = ml_dtypes.bfloat16

N_CORES = 8
D_IN = 1024
D_OUT = 1024
RANK = 4
ROWS_TOTAL = 4 * 8192
ROWS_PER_CORE = ROWS_TOTAL // N_CORES  # 4096
SCALING = 1.0 / RANK

P = 128            # partitions
CH = D_IN // P     # 8 feature chunks
SLAB = 512         # rows per pipeline step
NSLAB = ROWS_PER_CORE // SLAB  # 8
J = SLAB // P      # 4 row subtiles per slab (= row-tile lanes in mm2)
NO2 = D_OUT // 512  # 2 output column chunks of 512


def emit_lora(tc, xt_ap, at_ap, bt_ap, out_ap):
    """Emit the LoRA kernel IR for one core's shard.

    xt_ap : DRAM [NSLAB, P, CH, SLAB] bf16, xt[s, p, c, r] = x[s*SLAB+r, c*P+p]
    at_ap : DRAM [P, CH, P] bf16, at[p, c, 32g+r] = A[r, c*P+p] (g<4, r<4), 0 else
    bt_ap : DRAM [P, D_OUT] bf16, bt[32g+r, o] = SCALING * B[o, r] (g<4), 0 else
    out_ap: DRAM [NSLAB, NO2, P, J, 512] bf16,
            out[s, o2, p, j, o] = y[s*SLAB+j*P+p, o2*512+o]
    """
    import concourse.mybir as mybir

    nc = tc.nc
    f32 = mybir.dt.float32
    bf16 = mybir.dt.bfloat16
    ctx = tc._ctx  # ExitStack owned by caller

    consts = ctx.enter_context(tc.tile_pool(name="consts", bufs=1))
    xpool = ctx.enter_context(tc.tile_pool(name="xt", bufs=6))
    htpool = ctx.enter_context(tc.tile_pool(name="ht", bufs=3))
    opool = ctx.enter_context(tc.tile_pool(name="osb", bufs=3))
    # 8 PSUM banks total: htx 2 + o_r bufs (2,2,1,1) = 8.
    ps = ctx.enter_context(tc.tile_pool(name="ps", bufs=1, space="PSUM"))
    OR_BUFS = (2, 2, 1, 1)

    # Slab 0's load is split per chunk so the first mm1 matmul only waits
    # for 128 KiB; the tiny constants ride the SWDGE ring in parallel.
    xt0 = xpool.tile([P, CH, SLAB], bf16)
    for c in range(CH):
        nc.sync.dma_start(xt0[:, c, :], xt_ap[0, :, c, :])

    at_sb = consts.tile([P, CH, P], bf16)
    nc.gpsimd.dma_start(at_sb[:], at_ap[:])
    bt_sb = consts.tile([P, D_OUT], bf16)
    nc.gpsimd.dma_start(bt_sb[:], bt_ap[:])

    def emit_mm1_half(htX_ps, xt_sb, half):
        # htX[32g+r, m] += sum_f A[r, cP+f] * x^T[cP+f, m] for each replica
        # g; zero weight columns leave exact zeros between the replicas.
        # The two half-chains of a slab bracket the previous slab's mm2
        # waves, which write other PSUM banks (group check is skipped).
        for c in range(4 * half, 4 * half + 4):
            nc.tensor.matmul(
                htX_ps[:],
                lhsT=at_sb[:, c, :],
                rhs=xt_sb[:, c, :],
                start=(c == 0),
                stop=(c == CH - 1),
                skip_group_check=True,
            )

    def emit_wave(htX_sb, o2, s, eng_flip):
        o_ps = [
            ps.tile([P, 512], f32, name="o_ps", tag=f"o_r{r}", bufs=OR_BUFS[r])
            for r in range(J)
        ]
        for r in range(J):
            # out[m, o] = sum_r h^T[r, rP+m] * bt[r, o]; row-tile r of the
            # PE handles jtile r concurrently with the other three.
            nc.tensor.matmul(
                o_ps[r][:],
                lhsT=htX_sb[32 * r : 32 * r + RANK, r * P : (r + 1) * P],
                rhs=bt_sb[32 * r : 32 * r + RANK, o2 * 512 : (o2 + 1) * 512],
                start=True,
                stop=True,
                tile_position=(32 * r, 0),
            )
        # Evacuate the single-buffered banks (r=2,3) first, split across
        # ScalarE and VectorE so both free after ~one copy time.
        o_sb = opool.tile([P, J, 512], bf16, name="o_sb")
        for k, r in enumerate((2, 3, 0, 1)):
            dst = o_sb[:, r, :]
            if (k + eng_flip) % 2 == 0:
                nc.scalar.copy(dst, o_ps[r][:])
            else:
                nc.vector.tensor_copy(dst, o_ps[r][:])
        # Stores ride the SWDGE (gpsimd) ring so a store waiting on its
        # copy never head-of-line-blocks the HWDGE load ring.
        nc.gpsimd.dma_start(out_ap[s, o2], o_sb[:])

    pending = None  # (htX_sb, s) with both mm2 waves still to emit
    for s in range(NSLAB):
        if s == 0:
            xt_sb = xt0
        else:
            xt_sb = xpool.tile([P, CH, SLAB], bf16)
            nc.sync.dma_start(xt_sb[:], xt_ap[s])

        htX_ps = ps.tile([P, SLAB], f32, name="htX_ps", tag="htx", bufs=2)
        emit_mm1_half(htX_ps, xt_sb, 0)
        if pending is not None:
            emit_wave(pending[0], 0, pending[1], pending[1] % 2)
        emit_mm1_half(htX_ps, xt_sb, 1)

        htX_sb = htpool.tile([P, SLAB], bf16)
        if s % 2 == 0:
            nc.vector.tensor_copy(htX_sb[:], htX_ps[:])
        else:
            nc.scalar.copy(htX_sb[:], htX_ps[:])

        if pending is not None:
            emit_wave(pending[0], 1, pending[1], pending[1] % 2 + 1)
        pending = (htX_sb, s)

    emit_wave(pending[0], 0, pending[1], 0)
    emit_wave(pending[0], 1, pending[1], 1)


def build_nc():
    import concourse.mybir as mybir
    import concourse.tile as tile
    from concourse import bacc

    bf16 = mybir.dt.bfloat16
    nc = bacc.Bacc("TRN2", target_bir_lowering=False, debug=False)
    xt_d = nc.dram_tensor(
        "xt", [NSLAB, P, CH, SLAB], bf16, kind="ExternalInput"
    ).ap()
    at_d = nc.dram_tensor("at", [P, CH, P], bf16, kind="ExternalInput").ap()
    bt_d = nc.dram_tensor("bt", [P, D_OUT], bf16, kind="ExternalInput").ap()
    out_d = nc.dram_tensor(
        "out", [NSLAB, NO2, P, J, 512], bf16, kind="ExternalOutput"
    ).ap()

    with tile.TileContext(nc) as tc:
        with ExitStack() as ctx:
            tc._ctx = ctx
            emit_lora(tc, xt_d, at_d, bt_d, out_d)
    nc.compile()
    return nc


def host_prep_x(x2):
    """f32 [ROWS_TOTAL, D_IN] -> per-core bf16 [NSLAB, P, CH, SLAB]."""
    xb = x2.astype(BF16)
    shards = xb.reshape(N_CORES, NSLAB, SLAB, CH, P)
    return [
        np.ascontiguousarray(shards[i].transpose(0, 3, 2, 1))
        for i in range(N_CORES)
    ]


def host_prep_ab(lora_A, lora_B):
    # at[p, c, 32g+r] = A[r, c*P+p] for g in 0..3, zeros elsewhere
    a_pcr = (
        np.asarray(lora_A, dtype=np.float32)
        .T.reshape(CH, P, RANK)
        .transpose(1, 0, 2)
    )  # [P, CH, RANK]
    at = np.zeros((P, CH, P), dtype=np.float32)
    for g in range(4):
        at[:, :, 32 * g : 32 * g + RANK] = a_pcr
    # bt[32g+r, o] = SCALING * B[o, r], zeros elsewhere
    b_ro = np.asarray(lora_B, dtype=np.float32).T * SCALING  # [RANK, D_OUT]
    bt = np.zeros((P, D_OUT), dtype=np.float32)
    for g in range(4):
        bt[32 * g : 32 * g + RANK, :] = b_ro
    return np.ascontiguousarray(at.astype(BF16)), np.ascontiguousarray(
        bt.astype(BF16)
    )


def host_unpack_out(bufs):
    """Per-core bf16 [NSLAB, NO2, P, J, 512] -> f32 [4, 8192, D_OUT]."""
    full = np.stack([np.asarray(b) for b in bufs], axis=0)
    # [cores, s, o2, p, j, o] -> [cores, s, j, p, o2, o]; row = s*SLAB+j*P+p
    full = full.transpose(0, 1, 4, 3, 2, 5).reshape(ROWS_TOTAL, D_OUT)
    return full.astype(np.float32).reshape(4, 8192, D_OUT)


_NC_CACHE = {}


def kernel(x, lora_A, lora_B):
    from concourse.bass_utils import run_bass_kernel_spmd

    if "nc" not in _NC_CACHE:
        _NC_CACHE["nc"] = build_nc()
    nc = _NC_CACHE["nc"]

    x2 = np.ascontiguousarray(x, dtype=np.float32).reshape(ROWS_TOTAL, D_IN)
    xts = host_prep_x(x2)
    at, bt = host_prep_ab(lora_A, lora_B)
    in_maps = [{"xt": xts[i], "at": at, "bt": bt} for i in range(N_CORES)]
    res = run_bass_kernel_spmd(nc, in_maps, core_ids=list(range(N_CORES)))
    return host_unpack_out([res.results[i]["out"] for i in range(N_CORES)])
